# revision 1
# baseline (speedup 1.0000x reference)
"""Trainium2 Bass kernel for nn_DASAttentionGate — v2 (pixel-block-major).

Sharding: 8 cores = 4 samples x 2 H-halves (48 rows each); tiny AllReduces
for InstanceNorm/GroupNorm stats within sample pairs.

v2 layout: pixels enumerated in 8x16 blocks (sigma order): gather position
i = s*128 + part, slot s = q*6 + rblk, part = 16*(r%8) + (w%16), pixel
(r, w) = (8*rblk + r%8.., 16*q + w%16). Offset conv + bilinear math run
directly in this pixel-major layout (partition = pixel), so gather masks
need no DRAM staging; wrapped-16 gather indices are built with 8 tiny PE
fold matmuls. The quad table z4 is x-major so its DRAM write is 112 long
descriptors instead of ~28k short ones.
"""

import os
import sys

for _p in ("/opt/trn_rl_repo",):
    if os.path.isdir(_p) and _p not in sys.path:
        sys.path.insert(0, _p)

import numpy as np
import ml_dtypes

import concourse.bass as bass
import concourse.bacc as bacc
import concourse.tile as tile
from concourse import mybir
from concourse.bass_utils import run_bass_kernel_spmd

F32 = mybir.dt.float32
BF16 = mybir.dt.bfloat16
I16 = mybir.dt.int16
I32 = mybir.dt.int32
AF = mybir.ActivationFunctionType
OP = mybir.AluOpType

B, C, H, W = 4, 128, 96, 96
NCORES = 8
RPC = 48            # rows per core
GR, GCW = 64, 114   # hn grid rows (-8..55 local) / cols (-8..105)
ZY = 63             # z4 y blocks
NB = 112 * ZY       # 7056 quad blocks, x-major: idx = x*63 + y
NPIX = RPC * W
NS = 36             # sigma slots (8x16 blocks)
SPB = 12            # slots per gather band
NBAND = 3
NIDX = SPB * 128    # 1536 indices per gather call
EPS = 1e-5

_CACHE = {}


def _build_program():
    nc = bacc.Bacc("TRN2", target_bir_lowering=False, debug=False,
                   num_devices=NCORES)

    x_d = nc.dram_tensor("x_sh", [C, 66, GCW], BF16, kind="ExternalInput")
    vm_d = nc.dram_tensor("vrow", [C, GR], BF16, kind="ExternalInput")
    wf_d = nc.dram_tensor("wf", [C, 9, C], BF16, kind="ExternalInput")
    b1_d = nc.dram_tensor("b1c", [C, 1], F32, kind="ExternalInput")
    ow_d = nc.dram_tensor("ow", [C, 9, 18], BF16, kind="ExternalInput")
    ob_d = nc.dram_tensor("obr", [128, 18], F32, kind="ExternalInput")
    wd_d = nc.dram_tensor("wd", [C, 9, C], BF16, kind="ExternalInput")
    db_d = nc.dram_tensor("dbc", [C, 1], F32, kind="ExternalInput")
    gw_d = nc.dram_tensor("gwc", [C, 1], F32, kind="ExternalInput")
    gb_d = nc.dram_tensor("gbc", [C, 1], F32, kind="ExternalInput")
    id_d = nc.dram_tensor("idn", [128, 128], BF16, kind="ExternalInput")
    is_d = nc.dram_tensor("idsel", [128, 8, 16], F32, kind="ExternalInput")
    on_d = nc.dram_tensor("onec", [C, 1], F32, kind="ExternalInput")
    bt_d = nc.dram_tensor("baset", [128, NS, 18], F32, kind="ExternalInput")
    out_d = nc.dram_tensor("out_sh", [C, RPC, W], F32, kind="ExternalOutput")

    groups = [[0, 1], [2, 3], [4, 5], [6, 7]]

    with tile.TileContext(nc) as tc:
        with (
            tc.tile_pool(name="const", bufs=1) as constp,
            tc.tile_pool(name="xbuf", bufs=1) as xpool,
            tc.tile_pool(name="hbuf", bufs=1) as hpool,
            tc.tile_pool(name="mwork", bufs=1) as mpool,
            tc.tile_pool(name="sbig", bufs=1) as spool,
            tc.tile_pool(name="tbuf", bufs=1) as tpool,
            tc.tile_pool(name="gbuf", bufs=1) as gpool,
            tc.tile_pool(name="ps", bufs=6, space="PSUM") as psp,
            tc.tile_pool(name="dram", bufs=1, space="DRAM") as dramp,
        ):
            # ---- constants ----
            wf = constp.tile([C, 9, C], BF16, tag="wf")
            nc.sync.dma_start(wf[:], wf_d[:])
            b1 = constp.tile([C, 1], F32, tag="b1")
            nc.sync.dma_start(b1[:], b1_d[:])
            ow = constp.tile([C, 9, 18], BF16, tag="ow")
            nc.sync.dma_start(ow[:], ow_d[:])
            ob = constp.tile([128, 18], F32, tag="ob")
            nc.sync.dma_start(ob[:], ob_d[:])
            wd = constp.tile([C, 9, C], BF16, tag="wd")
            nc.sync.dma_start(wd[:], wd_d[:])
            db = constp.tile([C, 1], F32, tag="db")
            nc.sync.dma_start(db[:], db_d[:])
            gw = constp.tile([C, 1], F32, tag="gw")
            nc.sync.dma_start(gw[:], gw_d[:])
            gb = constp.tile([C, 1], F32, tag="gb")
            nc.sync.dma_start(gb[:], gb_d[:])
            idn = constp.tile([128, 128], BF16, tag="idn")
            nc.sync.dma_start(idn[:], id_d[:])
            idsel = constp.tile([128, 8, 16], F32, tag="idsel")
            nc.sync.dma_start(idsel[:], is_d[:])
            onec = constp.tile([C, 1], F32, tag="onec")
            nc.sync.dma_start(onec[:], on_d[:])
            baset = constp.tile([128, NS, 18], F32, tag="baset")
            nc.sync.dma_start(baset[:], bt_d[:])
            vm = constp.tile([C, GR], BF16, tag="vm")
            nc.sync.dma_start(vm[:], vm_d[:])

            # ---- conv1 (fused depthwise+pointwise, bf16) ----
            xs = xpool.tile([C, 66, GCW], BF16, tag="xs")
            nc.sync.dma_start(xs[:, 0:33, :], x_d[:, 0:33, :])
            nc.sync.dma_start(xs[:, 33:66, :], x_d[:, 33:66, :])

            hraw = hpool.tile([C, GR, 112], F32, tag="hraw")
            CH = 4
            for ch in range(GR // CH):
                gr0 = ch * CH
                pt = psp.tile([128, CH * 112], F32, tag="ps")
                for t in range(9):
                    ty, tx = t // 3, t % 3
                    rhs = xs[:, gr0 + ty:gr0 + ty + CH, tx:tx + 112]
                    nc.tensor.matmul(pt[:], wf[:, t, :], rhs,
                                     start=(t == 0), stop=(t == 8))
                nc.scalar.activation(
                    hraw[:, gr0:gr0 + CH, :].rearrange("p a b -> p (a b)"),
                    pt[:], AF.Identity, bias=b1[:])

            # ---- InstanceNorm stats + AllReduce ----
            valid = hraw[:, 8:56, 8:104]
            st = mpool.tile([C, 2], F32, tag="st")
            nc.vector.tensor_reduce(st[:, 0:1], valid, mybir.AxisListType.XY,
                                    OP.add)
            scratch = spool.tile([C, NPIX], F32, tag="scr")
            nc.scalar.activation(scratch[:].rearrange("p (a b) -> p a b", a=RPC),
                                 valid, AF.Square, accum_out=st[:, 1:2])
            cc_in = dramp.tile([C, 2], F32, tag="cci")
            cc_out = dramp.tile([C, 2], F32, tag="cco")
            nc.sync.dma_start(cc_in[:], st[:])
            nc.gpsimd.collective_compute(
                "AllReduce", OP.add, replica_groups=groups,
                ins=[cc_in[:].opt()], outs=[cc_out[:].opt()])
            stg = mpool.tile([C, 2], F32, tag="stg")
            nc.sync.dma_start(stg[:], cc_out[:])

            mom = mpool.tile([C, 2], F32, tag="mom")
            nc.vector.tensor_scalar(mom[:], stg[:], 1.0 / (H * W), None,
                                    OP.mult)
            var = mpool.tile([C, 1], F32, tag="var")
            nc.vector.tensor_tensor(var[:], mom[:, 0:1], mom[:, 0:1], OP.mult)
            nc.vector.tensor_tensor(var[:], mom[:, 1:2], var[:], OP.subtract)
            nc.vector.tensor_scalar(var[:], var[:], EPS, None, OP.add)
            rstd = mpool.tile([C, 1], F32, tag="rstd")
            nc.scalar.activation(rstd[:], var[:], AF.Sqrt)
            nc.vector.reciprocal(rstd[:], rstd[:])
            nbias = mpool.tile([C, 1], F32, tag="nbias")
            nc.vector.tensor_tensor(nbias[:], mom[:, 0:1], rstd[:], OP.mult)
            nc.vector.tensor_scalar(nbias[:], nbias[:], -1.0, None, OP.mult)

            # ---- hn bf16 (masked) + short (sigma order) ----
            hn = hpool.tile([C, GR, GCW], BF16, tag="hn")
            nc.scalar.activation(hn[:, :, 0:112], hraw[:], AF.Relu,
                                 bias=nbias[:], scale=rstd[:])
            vmb = vm[:].unsqueeze(2).broadcast_to((C, GR, 112))
            nc.vector.tensor_tensor(hn[:, :, 0:112], hn[:, :, 0:112], vmb,
                                    OP.mult)
            nc.vector.memset(hn[:, :, 0:8], 0.0)
            nc.vector.memset(hn[:, :, 104:GCW], 0.0)

            short = spool.tile([C, NS, 128], F32, tag="short")
            src_sigma = hraw[:, 8:56, 8:104].rearrange(
                "c (rb rr) (q p) -> c rb rr q p", rr=8, p=16).transpose(
                [0, 3, 1, 2, 4])
            nc.scalar.activation(
                short[:].rearrange("c (q rb) (rr p) -> c q rb rr p",
                                   rb=6, p=16),
                src_sigma, AF.Relu, bias=nbias[:], scale=rstd[:])

            # ---- hT4 quad assembly + z4 (x-major) ----
            z4 = dramp.tile([NB, 512], BF16, tag="z4")
            z4v = z4[:].rearrange("(x y) c -> x y c", y=ZY)
            YC = 9
            for c0 in range(0, ZY, YC):
                c1 = min(c0 + YC, ZY)
                ht4 = tpool.tile([112, YC, 4, 128], BF16, tag="ht4", bufs=2)
                for gr in range(c0, c1 + 1):
                    for jx in range(2):
                        pt = psp.tile([112, 128], F32, tag="ps")
                        nc.tensor.matmul(pt[:], hn[:, gr, jx:jx + 112],
                                         idn[:], start=True, stop=True)
                        if gr < c1:
                            nc.scalar.activation(
                                ht4[:, gr - c0, jx, :], pt[:], AF.Copy)
                        if gr - 1 >= c0:
                            nc.vector.tensor_copy(
                                ht4[:, gr - 1 - c0, 2 + jx, :], pt[:])
                nc.sync.dma_start(
                    z4v[:, c0:c1, :],
                    ht4[:, 0:c1 - c0, :, :].rearrange("x y j c -> x y (j c)"))

            # ---- per-pixel work tiles ----
            off_pm = mpool.tile([128, NS, 18], F32, tag="off_pm")
            pall = off_pm
            it32 = mpool.tile([128, NS, 18], I32, tag="it32")
            kf = mpool.tile([128, NS, 18], F32, tag="kf")
            gt = mpool.tile([128, NS, 18], F32, tag="gt")
            fr = mpool.tile([128, NS, 18], F32, tag="fr")
            un = mpool.tile([128, NS, 18], F32, tag="un")
            a_w = mpool.tile([128, NS, 9, 4, 2], BF16, tag="a_w")
            idxf = mpool.tile([128, NS, 9], F32, tag="idxf")
            idx_w = mpool.tile([128, NBAND, 9, 96], I16, tag="idx_w")
            d_sb = spool.tile([C, NS, 128], F32, tag="dsb")

            obv = ob[:].unsqueeze(1).broadcast_to((128, NS, 18))

            def band_masks(b):
                s0, s1 = SPB * b, SPB * (b + 1)
                # offset conv for the band's 12 blocks (stage windows
                # contiguous: matmul lhsT allows only one free dim)
                for s in range(s0, s1):
                    q, rblk = s // 6, s % 6
                    stg = tpool.tile([C, 9, 128], BF16, tag="ostg", bufs=2)
                    for t in range(9):
                        ty, tx = t // 3, t % 3
                        nc.vector.tensor_copy(
                            stg[:, t, :].rearrange("c (a b) -> c a b", a=8),
                            hn[:, 7 + 8 * rblk + ty:15 + 8 * rblk + ty,
                               7 + 16 * q + tx:23 + 16 * q + tx])
                    po = psp.tile([128, 18], F32, tag="ps")
                    for t in range(9):
                        nc.tensor.matmul(po[:], stg[:, t, :], ow[:, t, :],
                                         start=(t == 0), stop=(t == 8))
                    nc.scalar.activation(off_pm[:, s, :], po[:], AF.Copy)
                sl = slice(s0, s1)
                nc.vector.tensor_tensor(off_pm[:, sl], off_pm[:, sl],
                                        obv[:, sl], OP.add)
                # bilinear: floor/frac in interleaved (y,x) layout
                nc.vector.tensor_tensor(pall[:, sl], pall[:, sl],
                                        baset[:, sl], OP.add)
                nc.vector.tensor_copy(it32[:, sl], pall[:, sl])
                nc.vector.tensor_copy(kf[:, sl], it32[:, sl])
                nc.vector.tensor_tensor(gt[:, sl], kf[:, sl], pall[:, sl],
                                        OP.is_gt)
                nc.vector.tensor_tensor(kf[:, sl], kf[:, sl], gt[:, sl],
                                        OP.subtract)  # kf = floor
                nc.vector.tensor_tensor(fr[:, sl], pall[:, sl], kf[:, sl],
                                        OP.subtract)  # frac
                nc.vector.tensor_scalar(un[:, sl], fr[:, sl], -1.0, 1.0,
                                        OP.mult, OP.add)  # 1 - frac
                frv = fr[:].rearrange("c s (k two) -> c s k two", two=2)
                unv = un[:].rearrange("c s (k two) -> c s k two", two=2)
                uy, ux = unv[:, sl, :, 0], unv[:, sl, :, 1]
                ly, lx = frv[:, sl, :, 0], frv[:, sl, :, 1]

                def dup2(ap):
                    return ap.unsqueeze(3).broadcast_to((128, SPB, 9, 2))

                for j, (fa, fb) in enumerate(((uy, ux), (uy, lx),
                                              (ly, ux), (ly, lx))):
                    nc.vector.tensor_tensor(a_w[:, sl, :, j, :],
                                            dup2(fa), dup2(fb), OP.mult)
                # idx = fx*63 + fy, clamped
                kfv = kf[:].rearrange("c s (k two) -> c s k two", two=2)
                nc.vector.tensor_scalar(idxf[:, sl], kfv[:, sl, :, 1],
                                        float(ZY), None, OP.mult)
                nc.vector.tensor_tensor(idxf[:, sl], idxf[:, sl],
                                        kfv[:, sl, :, 0], OP.add)
                nc.vector.tensor_scalar(idxf[:, sl], idxf[:, sl], 0.0,
                                        float(NB - 1), OP.max, OP.min)
                # wrapped-16 fold via PE: col = s_local*8 + g8
                for g8 in range(8):
                    pf = psp.tile([16, SPB * 9], F32, tag="ps")
                    nc.tensor.matmul(
                        pf[:], idsel[:, g8, :], idxf[:, sl, :],
                        start=True, stop=True)
                    dest = idx_w[0:16, b, :, :].rearrange(
                        "p k (s g) -> p s k g", g=8)[:, :, :, g8]
                    nc.vector.tensor_copy(
                        dest, pf[:].rearrange("p (s k) -> p s k", k=9))
                nc.sync.dma_start(idx_w[16:32, b, :, :], idx_w[0:16, b, :, :])

            sampT = hpool.tile([C, 9, SPB, 128], BF16, tag="hraw")

            for b in range(NBAND):
                band_masks(b)
            for b in range(NBAND):
                for k in range(9):
                    g_t = gpool.tile([128, SPB, 4, 128], BF16, tag="g_t",
                                     bufs=2)
                    nc.gpsimd.dma_gather(
                        g_t[:].rearrange("p a b c -> p a (b c)"),
                        z4[:], idx_w[0:32, b, k, :],
                        NIDX, NIDX, 512, queue_num=0, single_packet=False)
                    gv = g_t[:].rearrange("p a b (c two) -> p a b c two",
                                          two=2)
                    for j in range(4):
                        av = a_w[:, SPB * b:SPB * (b + 1), k, j, :]
                        av = av.unsqueeze(2).broadcast_to((128, SPB, 64, 2))
                        nc.vector.tensor_tensor(gv[:, :, j], gv[:, :, j], av,
                                                OP.mult)
                    for t in range(SPB):
                        pt = psp.tile([128, 128], F32, tag="ps")
                        for j in range(4):
                            nc.tensor.matmul(pt[:], g_t[:, t, j, :], idn[:],
                                             start=(j == 0), stop=(j == 3))
                        nc.scalar.activation(sampT[:, k, t, :], pt[:],
                                             AF.Copy)
                for t in range(SPB):
                    pd = psp.tile([128, 128], F32, tag="ps")
                    for k in range(9):
                        nc.tensor.matmul(pd[:], wd[:, k, :],
                                         sampT[:, k, t, :],
                                         start=(k == 0), stop=(k == 8))
                    nc.scalar.activation(d_sb[:, SPB * b + t, :], pd[:],
                                         AF.Identity, bias=db[:])

            # ---- GroupNorm stats ----
            gst = mpool.tile([C, 2], F32, tag="gst")
            nc.vector.tensor_reduce(gst[:, 0:1], d_sb[:],
                                    mybir.AxisListType.XY, OP.add)
            nc.scalar.activation(scratch[:].rearrange("p (a b) -> p a b",
                                                      a=NS),
                                 d_sb[:], AF.Square, accum_out=gst[:, 1:2])
            pg = psp.tile([1, 2], F32, tag="ps")
            nc.tensor.matmul(pg[:], onec[:], gst[:], start=True, stop=True)
            gred = mpool.tile([1, 2], F32, tag="gred")
            nc.scalar.activation(gred[:], pg[:], AF.Copy)
            ccg_in = dramp.tile([1, 2], F32, tag="ccgi")
            ccg_out = dramp.tile([1, 2], F32, tag="ccgo")
            nc.sync.dma_start(ccg_in[:], gred[:])
            nc.gpsimd.collective_compute(
                "AllReduce", OP.add, replica_groups=groups,
                ins=[ccg_in[:].opt()], outs=[ccg_out[:].opt()])
            gsc = mpool.tile([1, 2], F32, tag="gsc")
            nc.sync.dma_start(gsc[:], ccg_out[:])
            gall = mpool.tile([128, 2], F32, tag="gall")
            nc.gpsimd.partition_broadcast(gall[:], gsc[:], 128)

            gmom = mpool.tile([C, 2], F32, tag="gmom")
            nc.vector.tensor_scalar(gmom[:], gall[:], 1.0 / (C * H * W),
                                    None, OP.mult)
            gvar = mpool.tile([C, 1], F32, tag="gvar")
            nc.vector.tensor_tensor(gvar[:], gmom[:, 0:1], gmom[:, 0:1],
                                    OP.mult)
            nc.vector.tensor_tensor(gvar[:], gmom[:, 1:2], gvar[:],
                                    OP.subtract)
            nc.vector.tensor_scalar(gvar[:], gvar[:], EPS, None, OP.add)
            grstd = mpool.tile([C, 1], F32, tag="grstd")
            nc.scalar.activation(grstd[:], gvar[:], AF.Sqrt)
            nc.vector.reciprocal(grstd[:], grstd[:])
            sc2 = mpool.tile([C, 1], F32, tag="sc2")
            nc.vector.tensor_tensor(sc2[:], gw[:], grstd[:], OP.mult)
            bi2 = mpool.tile([C, 1], F32, tag="bi2")
            nc.vector.tensor_tensor(bi2[:], gmom[:, 0:1], sc2[:], OP.mult)
            nc.vector.tensor_tensor(bi2[:], gb[:], bi2[:], OP.subtract)

            # ---- gate + residual + un-permute ----
            nc.scalar.activation(scratch[:].rearrange("p (a b) -> p a b",
                                                      a=NS),
                                 d_sb[:], AF.Sigmoid, bias=bi2[:],
                                 scale=sc2[:])
            nc.vector.tensor_scalar(scratch[:], scratch[:], 1.0, None,
                                    OP.add)
            go = spool.tile([C, RPC, W], F32, tag="dsb")
            gov = go[:].rearrange("c (rb rr) (q p) -> c rb rr q p",
                                  rr=8, p=16).transpose([0, 3, 1, 2, 4])
            nc.vector.tensor_tensor(
                gov,
                scratch[:].rearrange("c (q rb rr p) -> c q rb rr p",
                                     q=6, rb=6, rr=8),
                short[:].rearrange("c (q rb) (rr p) -> c q rb rr p",
                                   rb=6, p=16),
                OP.mult)
            nc.sync.dma_start(out_d[:], go[:])

    nc.compile()
    return nc


def _prep_inputs(inputs):
    x = np.asarray(inputs["x"], np.float32)
    dw_w = np.asarray(inputs["dw_w"], np.float32)
    dw_b = np.asarray(inputs["dw_b"], np.float32)
    pw_w = np.asarray(inputs["pw_w"], np.float32)
    pw_b = np.asarray(inputs["pw_b"], np.float32)
    off_w = np.asarray(inputs["off_w"], np.float32)
    off_b = np.asarray(inputs["off_b"], np.float32)
    de_w = np.asarray(inputs["de_w"], np.float32)
    de_b = np.asarray(inputs["de_b"], np.float32)
    gn_w = np.asarray(inputs["gn_w"], np.float32)
    gn_b = np.asarray(inputs["gn_b"], np.float32)

    bf = ml_dtypes.bfloat16
    dwt = dw_w.reshape(C, 9)
    wf = np.ascontiguousarray(
        (pw_w.T[None, :, :] * dwt.T[:, :, None]).transpose(1, 0, 2)
    ).astype(bf)                                     # [c, t, o]
    b1 = (pw_w @ dw_b + pw_b).astype(np.float32).reshape(C, 1)
    ow = np.ascontiguousarray(
        off_w.reshape(18, C, 9).transpose(1, 2, 0)).astype(bf)
    obr = np.ascontiguousarray(
        np.broadcast_to(off_b[None, :], (128, 18))).astype(np.float32)
    wdm = np.ascontiguousarray(
        de_w.reshape(C, C, 9).transpose(1, 2, 0)).astype(bf)
    dbc = de_b.reshape(C, 1).astype(np.float32)
    gwc = gn_w.reshape(C, 1).astype(np.float32)
    gbc = gn_b.reshape(C, 1).astype(np.float32)
    idn = np.eye(128, dtype=bf)
    idsel = np.zeros((128, 8, 16), np.float32)
    for p in range(128):
        idsel[p, p // 16, p % 16] = 1.0
    onec = np.ones((C, 1), np.float32)
    # base table [128, 36, 18]: interleaved (y, x) per tap
    rr_ = np.arange(128) // 16
    p_ = np.arange(128) % 16
    ks = np.arange(9)
    kyv = ks // 3 - 1
    kxv = ks % 3 - 1
    baset = np.zeros((128, NS, 18), np.float32)
    for s in range(NS):
        q, rblk = s // 6, s % 6
        baset[:, s, 0::2] = (8 * rblk + rr_)[:, None] + 8 + kyv[None, :]
        baset[:, s, 1::2] = (16 * q + p_)[:, None] + 8 + kxv[None, :]

    in_maps = []
    for core in range(NCORES):
        b = core // 2
        r0 = (core % 2) * RPC
        xp = np.zeros((C, 66, GCW), np.float32)
        glo, ghi = max(0, r0 - 9), min(H, r0 + 57)
        xp[:, glo - (r0 - 9):ghi - (r0 - 9), 9:105] = x[b, :, glo:ghi, :]
        vrow = np.zeros((C, GR), bf)
        vlo, vhi = max(0, r0 - 8), min(H, r0 + 56)
        vrow[:, vlo - (r0 - 8):vhi - (r0 - 8)] = bf(1.0)
        in_maps.append({
            "x_sh": xp.astype(bf), "vrow": vrow, "wf": wf, "b1c": b1,
            "ow": ow, "obr": obr, "wd": wdm, "dbc": dbc, "gwc": gwc,
            "gbc": gbc, "idn": idn, "idsel": idsel, "onec": onec,
            "baset": baset,
        })
    return in_maps


def get_program():
    if "nc" not in _CACHE:
        _CACHE["nc"] = _build_program()
    return _CACHE["nc"]


def kernel(**inputs):
    nc = get_program()
    in_maps = _prep_inputs(inputs)
    res = run_bass_kernel_spmd(nc, in_maps, core_ids=list(range(NCORES)))
    out = np.empty((B, C, H, W), np.float32)
    rr_ = np.arange(128) // 16
    p_ = np.arange(128) % 16
    for core in range(NCORES):
        b = core // 2
        r0 = (core % 2) * RPC
        o = res.results[core]["out_sh"]
        out[b, :, r0:r0 + RPC, :] = o
    return out

